# revision 37
# baseline (speedup 1.0000x reference)
"""LocalVarianceMap Trainium2 kernel (bf16 pipeline).

reference:
  lum  = mean over channel of x            (B,1,H,W)
  mean = 7x7 'same' box mean of lum ; sqm = same of lum^2
  out  = sqm - mean^2

Full input x: (16, 3, 1024, 1024) fp32. Data-parallel over batch:
8 NeuronCores x 2 images each. Host casts x to bf16 (halves HBM read
traffic); device computes in bf16 with fp32 PSUM accumulation; output
stored bf16 and upcast on host. Verified rel err ~1.1e-2 < 2e-2 gate.

Per-core math (l = c0+c1+c2 = 3*lum, s = l^2):
  S1 = 49-box-sum(l) = 147*mean       -> m2 = (S1/7)^2 = 441*mean^2
  S2 = m2 - 49-box-sum(s) = 441*(mean^2 - sqm)
  V  = S2 * (-1/441) = var

Horizontal 7-sum decomposition (per row, zero-padded):
  a[t] = z[t] + z[t+1]            (DVE tensor_tensor, bf16 2x mode)
  b[t] = a[t] + a[t+2]            (4-sum)
  h[j] = b[j] + a[j+4] + z[j+6]   (3 shifted matmuls accumulating in PSUM)
Vertical 7-sum is the band stationary of those same matmuls; the sq path
uses negated bands and starts its PSUM group with an identity-block
matmul of m2 so no separate subtract op is needed.

Tile pipeline (18 tiles/core, 128-row input tiles, 6-row halo), with
emission skew so each engine queue head always has a ready tile:
  SP: load ch0 | Pool/Q7: one CCE-accumulate DMA for ch1+ch2 (stride-0
  out AP), store | DVE: sq, a, b | PE: 3 S1 matmuls (N=1024) -> ACT: m2
  -> PE: I+3 S2 matmuls | ACT: V = Copy(S2 * -1/441)
"""

import sys

if "/opt/trn_rl_repo" not in sys.path:
    sys.path.insert(0, "/opt/trn_rl_repo")

import numpy as np
from contextlib import ExitStack

import concourse.bass as bass
import concourse.bacc as bacc
import concourse.tile as tile
from concourse import mybir

H = 1024
W = 1024
C = 3
PER_CORE_B = 2
N_CORES = 8
K7 = 7

LZ = 1040                 # per-path region width inside the combined tile
ZW = 2 * LZ               # combined [pad3|lum 1024|zeros][pad3|sq 1024|zeros]
LUM0 = 3                  # lum col w at Z[:, LUM0 + w]
SQ0 = LZ + 3              # sq col w at Z[:, SQ0 + w]
NRING = 14


def _tiles():
    specs = []
    for b in range(PER_CORE_B):
        specs.append(dict(b=b, r0=0, nr=128, K=128, M=125, out_r0=0, w=0))
        for t in range(1, 8):
            specs.append(
                dict(b=b, r0=122 * t, nr=128, K=128, M=122, out_r0=122 * t + 3, w=1)
            )
        specs.append(dict(b=b, r0=976, nr=48, K=48, M=45, out_r0=979, w=2))
    return specs


def band_weights() -> np.ndarray:
    """Seven [128,128] blocks: +W0|+W1|+W2|-W0|-W1|-W2|+I."""
    wb = np.zeros((128, 7 * 128), np.float32)
    for m in range(125):
        for k in range(max(m - 3, 0), m + 4):
            wb[k, m] = 1.0
    for m in range(122):
        for k in range(m, m + 7):
            wb[k, 128 + m] = 1.0
    for m in range(45):
        for k in range(m, min(m + 7, 48)):
            wb[k, 256 + m] = 1.0
    wb[:, 384:768] = -wb[:, 0:384]
    for m in range(128):
        wb[m, 768 + m] = 1.0
    return wb


def build_nc(finalize: bool = True) -> bass.Bass:
    nc = bacc.Bacc("TRN2", target_bir_lowering=False)
    bf16 = mybir.dt.bfloat16
    f32 = mybir.dt.float32

    x = nc.dram_tensor("x", [PER_CORE_B, C, H, W], bf16, kind="ExternalInput")
    wbt = nc.dram_tensor("wb", [128, 7 * 128], bf16, kind="ExternalInput")
    y = nc.dram_tensor("y", [PER_CORE_B, 1, H, W], bf16, kind="ExternalOutput")

    inv7 = float(np.float32(1.0) / np.float32(7.0))
    ninv441 = -float(np.float32(1.0) / np.float32(441.0))

    specs = _tiles()
    NT = len(specs)

    with tile.TileContext(nc) as tc, ExitStack() as ctx:
        cpool = ctx.enter_context(tc.tile_pool(name="const", bufs=1))
        apool = ctx.enter_context(tc.tile_pool(name="atree", bufs=7))
        bpool = ctx.enter_context(tc.tile_pool(name="btree", bufs=7))
        xcpool = ctx.enter_context(tc.tile_pool(name="xc", bufs=5))
        mpool = ctx.enter_context(tc.tile_pool(name="m2", bufs=6))
        vpool = ctx.enter_context(tc.tile_pool(name="vout", bufs=4))
        p1pool = ctx.enter_context(tc.tile_pool(name="ps1", bufs=2, space="PSUM"))
        p2pool = ctx.enter_context(tc.tile_pool(name="ps2", bufs=2, space="PSUM"))

        WB = cpool.tile([128, 7 * 128], bf16)
        nc.sync.dma_start(out=WB[:], in_=wbt[:, :])

        # Persistent combined lum|sq ring; zero the pad columns once.
        zring = [
            cpool.tile([128, ZW], bf16, tag=f"zr{i}", name=f"zr{i}")
            for i in range(NRING)
        ]
        for t_ in zring:
            nc.gpsimd.memset(t_[:, 0:LUM0], 0.0)
            nc.gpsimd.memset(t_[:, LUM0 + W : SQ0], 0.0)
            nc.gpsimd.memset(t_[:, SQ0 + W : ZW], 0.0)

        As, Bs, M2s, S1s, S2s, Vs, XCs = {}, {}, {}, {}, {}, {}, {}

        def st_load(t):
            sp = specs[t]
            nr = sp["nr"]
            XC = xcpool.tile([128, 3 * W], bf16, tag="XC", name=f"XC_{t}")
            for c in range(3):
                nc.sync.dma_start(
                    out=XC[0:nr, c * W : (c + 1) * W],
                    in_=x[sp["b"], c, sp["r0"] : sp["r0"] + nr, :],
                )
            XCs[t] = XC

        def st_add1(t):
            nr = specs[t]["nr"]
            z = zring[t % NRING]
            XC = XCs[t]
            nc.vector.tensor_tensor(
                z[0:nr, LUM0 : LUM0 + W],
                XC[0:nr, 0:W],
                XC[0:nr, W : 2 * W],
                op=mybir.AluOpType.add,
            )

        def st_add2(t):
            nr = specs[t]["nr"]
            z = zring[t % NRING]
            XC = XCs.pop(t)
            nc.vector.tensor_tensor(
                z[0:nr, LUM0 : LUM0 + W],
                z[0:nr, LUM0 : LUM0 + W],
                XC[0:nr, 2 * W : 3 * W],
                op=mybir.AluOpType.add,
            )

        def st_sq(t):
            nr = specs[t]["nr"]
            z = zring[t % NRING]
            nc.scalar.activation(
                z[0:nr, SQ0 : SQ0 + W],
                z[0:nr, LUM0 : LUM0 + W],
                mybir.ActivationFunctionType.Square,
            )

        def st_treeL(t):
            nr = specs[t]["nr"]
            z = zring[t % NRING]
            A = apool.tile([128, ZW], bf16, tag="A", name=f"A_{t}")
            B = bpool.tile([128, ZW], bf16, tag="B", name=f"B_{t}")
            nc.vector.tensor_tensor(
                A[0:nr, 0 : ZW - 1],
                z[0:nr, 0 : ZW - 1],
                z[0:nr, 1:ZW],
                op=mybir.AluOpType.add,
            )
            nc.vector.tensor_tensor(
                B[0:nr, 0 : ZW - 3],
                A[0:nr, 0 : ZW - 3],
                A[0:nr, 2 : ZW - 1],
                op=mybir.AluOpType.add,
            )
            As[t] = A
            Bs[t] = B

        def st_treeS(t):
            pass

        def st_mm1(t):
            sp = specs[t]
            K, M, wsel = sp["K"], sp["M"], sp["w"]
            z = zring[t % NRING]
            A, B = As[t], Bs[t]
            S1 = p1pool.tile([128, W], f32, tag="S1", name=f"S1_{t}")
            wpos = 128 * wsel
            for n0 in (0, 512):
                for src, off, st, sp_ in (
                    (B, 0, True, False),
                    (A, 4, False, False),
                    (z, 6, False, True),
                ):
                    nc.tensor.matmul(
                        S1[0:M, n0 : n0 + 512],
                        WB[0:K, wpos : wpos + M],
                        src[0:K, n0 + off : n0 + off + 512],
                        start=st,
                        stop=sp_,
                    )
            S1s[t] = S1

        def st_m2(t):
            M = specs[t]["M"]
            m2 = mpool.tile([128, W], bf16, tag="m2", name=f"m2_{t}")
            nc.scalar.activation(
                m2[0:M, :],
                S1s.pop(t)[0:M, :],
                mybir.ActivationFunctionType.Square,
                scale=inv7,
            )
            M2s[t] = m2

        def st_mm2(t):
            sp = specs[t]
            K, M, wsel = sp["K"], sp["M"], sp["w"]
            z = zring[t % NRING]
            A, B = As.pop(t), Bs.pop(t)
            m2 = M2s.pop(t)
            S2 = p2pool.tile([128, W], f32, tag="S2", name=f"S2_{t}")
            wneg = 128 * (3 + wsel)
            for n0 in (0, 512):
                nc.tensor.matmul(
                    S2[0:M, n0 : n0 + 512],
                    WB[0:M, 768 : 768 + M],
                    m2[0:M, n0 : n0 + 512],
                    start=True,
                    stop=False,
                )
            for n0 in (0, 512):
                for src, off, sp_ in ((B, 0, False), (A, 4, False), (z, 6, True)):
                    nc.tensor.matmul(
                        S2[0:M, n0 : n0 + 512],
                        WB[0:K, wneg : wneg + M],
                        src[0:K, LZ + n0 + off : LZ + n0 + off + 512],
                        start=False,
                        stop=sp_,
                    )
            S2s[t] = S2

        def st_v(t):
            M = specs[t]["M"]
            V = vpool.tile([128, W], bf16, tag="V", name=f"V_{t}")
            nc.scalar.activation(
                V[0:M, :],
                S2s.pop(t)[0:M, :],
                mybir.ActivationFunctionType.Copy,
                scale=ninv441,
            )
            Vs[t] = V

        def st_out(t):
            sp = specs[t]
            M = sp["M"]
            nc.gpsimd.dma_start(
                out=y[sp["b"], 0, sp["out_r0"] : sp["out_r0"] + M, :],
                in_=Vs.pop(t)[0:M, :],
            )

        for i in range(NT + 11):
            if i < NT:
                st_load(i)
            if 2 <= i < NT + 2:
                st_add1(i - 2)
            if 5 <= i < NT + 5:
                st_treeL(i - 5)
            if 3 <= i < NT + 3:
                st_add2(i - 3)
            if 9 <= i < NT + 9:
                st_mm2(i - 9)
            if 6 <= i < NT + 6:
                with tc.high_priority(offset=100):
                    st_mm1(i - 6)
            if 7 <= i < NT + 7:
                with tc.high_priority():
                    st_m2(i - 7)
            if 4 <= i < NT + 4:
                st_sq(i - 4)
            if 10 <= i < NT + 10:
                st_v(i - 10)
            if 11 <= i < NT + 11:
                st_out(i - 11)

    if finalize:
        nc.finalize()
    return nc


def kernel(x, kernel_size):
    assert int(kernel_size) == K7
    import ml_dtypes

    x = np.asarray(x)
    B = x.shape[0]
    assert x.shape == (B, C, H, W) and B == PER_CORE_B * N_CORES
    xb = np.ascontiguousarray(x.astype(ml_dtypes.bfloat16))

    from concourse.bass_utils import run_bass_kernel_spmd

    nc = build_nc()
    wb = band_weights().astype(ml_dtypes.bfloat16)
    in_maps = [
        {"x": xb[i * PER_CORE_B : (i + 1) * PER_CORE_B], "wb": wb}
        for i in range(N_CORES)
    ]
    res = run_bass_kernel_spmd(nc, in_maps, list(range(N_CORES)))
    y = np.concatenate([res.results[i]["y"] for i in range(N_CORES)], axis=0)
    return y.astype(np.float32)
